# revision 11
# baseline (speedup 1.0000x reference)
"""Multi-head attention (B=2, N=2048, EMB=1024, H=16, hd=64) on 8 TRN2 NeuronCores.

Sharding: tensor-parallel over heads. Each core owns 2 heads: it gets the
W_qkv columns (k|q|v sections) and W_out rows for those heads, computes
QKV projection + attention + its partial output projection, and the host
sums the 8 partials (the "all-reduce") and adds b_out.

Device kernel layout (per core), all matmuls bf16 with fp32 PSUM accumulation:
  - x is pre-transposed on host to xT [EMB, TOK] so the embedding dim lands on
    SBUF partitions (matmul contraction dim).
  - QKV^T is produced in [dims, tokens] layout: lhsT = W chunk, rhs = xT chunk.
    K/Q sections stay transposed ([hd, tok]) for the scores matmul; the V
    section is staged transposed then PE-transposed into [tok, hd(+ones)]
    tiles (V_aug) for the attn@V matmul.
  - scores^T chunks [k_tok 128, q 512] per head via row-packed (K=64) matmuls,
    exp on ScalarE straight out of PSUM (scale=1/8 folded in, no max
    subtraction needed: scores ~ N(0,1)), bf16 expT.
  - attn@V: lhsT = V_aug [k_tok 128, 65] (col 64 = ones -> row 64 of the
    output accumulates the softmax denominator), accumulated over 16 k chunks.
  - normalize: reciprocal of the denominator row, partition-broadcast DMA,
    multiply on VectorE into A_norm [att 128, tok] bf16.
  - out projection: lhsT = A_norm chunk [128, 128], rhs = W_out shard
    [128, 512], PSUM -> SBUF -> DRAM partial [TOK, EMB] fp32.
"""

import os

import numpy as np
import ml_dtypes

B = 2
N = 2048
EMB = 1024
TOK = B * N  # 4096
HD = 64
H_PER_CORE = 2
DIMS = 3 * H_PER_CORE * HD  # 384 qkv cols per core
ATT_LOCAL = H_PER_CORE * HD  # 128
P = 128
EC = EMB // P  # 8 embedding chunks
TCQ = TOK // 512  # 8 token chunks for the qkv projection
KCH = N // P  # 16 key chunks per batch
QQ = N // 512  # 4 query quarters per batch
SCALE = HD ** -0.5

_CACHE = {}
LAST = {}


def _build_graph():
    from concourse import bacc, mybir
    import concourse.tile as tile

    nc = bacc.Bacc(
        "TRN2", target_bir_lowering=False, debug=False, num_devices=8
    )
    dt = mybir.dt
    xT = nc.dram_tensor("xT", [EMB, TOK], dt.bfloat16, kind="ExternalInput")
    wqkv = nc.dram_tensor("wqkv", [EMB, DIMS], dt.bfloat16, kind="ExternalInput")
    bqkv = nc.dram_tensor("bqkv", [DIMS], dt.float32, kind="ExternalInput")
    wout = nc.dram_tensor("wout", [ATT_LOCAL, EMB], dt.bfloat16, kind="ExternalInput")
    out = nc.dram_tensor("out", [TOK, EMB], dt.float32, kind="ExternalOutput")

    dbg = {}
    if os.environ.get("KERNEL_DEBUG") == "1":
        dbg["kq"] = nc.dram_tensor(
            "dbg_kq", [2, P, TOK], dt.bfloat16, kind="ExternalOutput"
        )
        dbg["vt"] = nc.dram_tensor(
            "dbg_vt", [P, TOK], dt.float32, kind="ExternalOutput"
        )
        dbg["vaug"] = nc.dram_tensor(
            "dbg_vaug", [P, B * H_PER_CORE * KCH * (HD + 1)], dt.bfloat16,
            kind="ExternalOutput",
        )
        dbg["exp"] = nc.dram_tensor(
            "dbg_exp", [P, 1024], dt.bfloat16, kind="ExternalOutput"
        )
        dbg["psa"] = nc.dram_tensor(
            "dbg_psa", [2, HD + 1, 512], dt.float32, kind="ExternalOutput"
        )
        dbg["rrep"] = nc.dram_tensor(
            "dbg_rrep", [2, HD, 512], dt.float32, kind="ExternalOutput"
        )
        dbg["anorm"] = nc.dram_tensor(
            "dbg_anorm", [P, TOK], dt.bfloat16, kind="ExternalOutput"
        )

    with tile.TileContext(nc) as tc:
        _emit(tc, nc, xT, wqkv, bqkv, wout, out, dbg)
    nc.compile()
    return nc


def _emit(tc, nc, xT, wqkv, bqkv, wout, out, dbg=None):
    dbg = dbg or {}
    from contextlib import ExitStack
    import concourse.bass as bass
    from concourse import mybir
    from concourse.masks import make_identity

    dt = mybir.dt
    f32, bf16 = dt.float32, dt.bfloat16
    Exp = mybir.ActivationFunctionType.Exp

    with ExitStack() as ctx:
        consts = ctx.enter_context(tc.tile_pool(name="consts", bufs=1))
        xt_pool = ctx.enter_context(tc.tile_pool(name="xt", bufs=2))
        persist = ctx.enter_context(tc.tile_pool(name="persist", bufs=1))
        expp = ctx.enter_context(tc.tile_pool(name="expp", bufs=6))
        small = ctx.enter_context(tc.tile_pool(name="small", bufs=2))
        dram_p = ctx.enter_context(tc.tile_pool(name="dram_p", bufs=2, space="DRAM"))
        outst = ctx.enter_context(tc.tile_pool(name="outst", bufs=3))
        ps_scores = ctx.enter_context(
            tc.tile_pool(name="ps_scores", bufs=2, space="PSUM")
        )
        ps_att = ctx.enter_context(tc.tile_pool(name="ps_att", bufs=2, space="PSUM"))
        ps_small = ctx.enter_context(
            tc.tile_pool(name="ps_small", bufs=2, space="PSUM")
        )

        # ---- constants / persistent tiles ----
        # warm up the exp table set as early as possible (one-time ~2.7us)
        warm = consts.tile([1, 8], f32, tag="warm")
        nc.vector.memset(warm, 0.0)
        nc.scalar.activation(out=warm, in_=warm, func=Exp, scale=1.0)

        w_sb = consts.tile([P, EC, DIMS], bf16, tag="w_sb")
        for e in range(EC):
            nc.sync.dma_start(out=w_sb[:, e, :], in_=wqkv[e * P : (e + 1) * P, :])
        bias_sb = consts.tile([P, 3], f32, tag="bias_sb")
        nc.sync.dma_start(out=bias_sb, in_=bqkv[:].rearrange("(c p) -> p c", p=P))
        wout_sb = consts.tile([P, EMB], bf16, tag="wout_sb")
        nc.sync.dma_start(out=wout_sb, in_=wout[:, :])
        ident = consts.tile([P, P], f32, tag="ident")
        make_identity(nc, ident)

        k_sb = persist.tile([P, TOK], bf16, tag="k_sb")
        q_sb = persist.tile([P, TOK], bf16, tag="q_sb")
        vt_sb = persist.tile([P, TOK], f32, tag="vt_sb")
        vaug = persist.tile([P, B, H_PER_CORE, KCH, HD + 1], bf16, tag="vaug")
        anorm = persist.tile([P, TOK], bf16, tag="anorm")
        # ones column of V_aug (the softmax denominator accumulator row)
        nc.vector.memset(vaug[:, :, :, :, HD : HD + 1], 1.0)

        qkv_dst = (k_sb, q_sb, vt_sb)

        def qkv_chunk(t):
            # tokens t*512 .. (t+1)*512
            xt = xt_pool.tile([P, EC, 512], bf16, tag="xt")
            for e in range(EC):
                nc.sync.dma_start(
                    out=xt[:, e, :], in_=xT[e * P : (e + 1) * P, bass.ts(t, 512)]
                )
            for d in range(3):
                ps = ps_small.tile([P, 512], f32, tag="ps_small")
                for e in range(EC):
                    nc.tensor.matmul(
                        ps,
                        lhsT=w_sb[:, e, d * P : (d + 1) * P],
                        rhs=xt[:, e, :],
                        start=(e == 0),
                        stop=(e == EC - 1),
                    )
                nc.vector.tensor_scalar_add(
                    out=qkv_dst[d][:, bass.ts(t, 512)],
                    in0=ps,
                    scalar1=bias_sb[:, d : d + 1],
                )

        def vtrans(b):
            # fill vaug[:, b, h, i, 0:64] = V[tok chunk i, head h] for batch b
            for i in range(KCH):
                base = b * N + i * P
                for h in range(H_PER_CORE):
                    ps = ps_small.tile([P, 512], f32, tag="ps_small")
                    nc.tensor.transpose(
                        ps[:, 0:HD],
                        in_=vt_sb[h * HD : (h + 1) * HD, base : base + P],
                        identity=ident[h * HD : (h + 1) * HD, h * HD : (h + 1) * HD],
                    )
                    nc.vector.tensor_copy(
                        out=vaug[:, b, h, i, 0:HD], in_=ps[:, 0:HD]
                    )

        def attention_unit(b, qq):
            # query tokens qbase .. qbase+512 of batch b, both heads
            qbase = b * N + qq * 512
            ps_a = [
                ps_att.tile([HD + 1, 512], f32, tag="ps_att", name=f"ps_a{b}_{qq}_{h}")
                for h in range(H_PER_CORE)
            ]
            for i in range(KCH):
                kbase = b * N + i * P
                ps_s = ps_scores.tile([P, 1024], f32, tag="ps_s")
                for h in range(H_PER_CORE):
                    nc.tensor.matmul(
                        ps_s[:, h * 512 : (h + 1) * 512],
                        lhsT=k_sb[h * HD : (h + 1) * HD, kbase : kbase + P],
                        rhs=q_sb[h * HD : (h + 1) * HD, qbase : qbase + 512],
                        start=True,
                        stop=True,
                    )
                ex = expp.tile([P, 1024], bf16, tag="expT")
                nc.scalar.activation(out=ex, in_=ps_s, func=Exp, scale=SCALE)
                if "exp" in dbg and (b, qq, i) == (0, 0, 0):
                    nc.sync.dma_start(out=dbg["exp"][:, :], in_=ex)
                for h in range(H_PER_CORE):
                    nc.tensor.matmul(
                        ps_a[h],
                        lhsT=vaug[:, b, h, i, :],
                        rhs=ex[:, h * 512 : (h + 1) * 512],
                        start=(i == 0),
                        stop=(i == KCH - 1),
                    )
            for h in range(H_PER_CORE):
                if "psa" in dbg and (b, qq) == (0, 0):
                    psa_st = small.tile([HD + 1, 512], f32, tag="psa_st")
                    nc.vector.tensor_copy(out=psa_st, in_=ps_a[h])
                    nc.sync.dma_start(out=dbg["psa"][h], in_=psa_st)
                rc = small.tile([HD + 1, 512], f32, tag="recip")
                nc.vector.reciprocal(out=rc[HD : HD + 1, :], in_=ps_a[h][HD : HD + 1, :])
                # partition-broadcast via a DRAM bounce: SBUF-source DMAs
                # reject partition-step-0 APs, DRAM-source ones allow it.
                rdram = dram_p.tile([1, 512], f32, tag="rdram")
                nc.sync.dma_start(out=rdram[:, :], in_=rc[HD : HD + 1, :])
                rrep = small.tile([HD, 512], f32, tag="rrep")
                nc.sync.dma_start(
                    out=rrep, in_=rdram[0:1, :].to_broadcast((HD, 512))
                )
                if "rrep" in dbg and (b, qq) == (0, 0):
                    nc.sync.dma_start(out=dbg["rrep"][h], in_=rrep)
                if h == 0:
                    nc.vector.tensor_mul(
                        out=anorm[0:HD, qbase : qbase + 512],
                        in0=ps_a[h][0:HD, :],
                        in1=rrep,
                    )
                else:
                    # VectorE lanes cannot shift partitions; go through a
                    # partition-0 temp and DMA into partitions 64..127.
                    tmp = small.tile([HD, 512], bf16, tag="anorm_tmp")
                    nc.vector.tensor_mul(out=tmp, in0=ps_a[h][0:HD, :], in1=rrep)
                    nc.sync.dma_start(
                        out=anorm[HD : 2 * HD, qbase : qbase + 512], in_=tmp
                    )

        def outproj_unit(b, qq):
            qbase = b * N + qq * 512
            for tci in range(4):
                tok0 = qbase + tci * P
                for e2 in range(2):
                    ps = ps_small.tile([P, 512], f32, tag="ps_small")
                    nc.tensor.matmul(
                        ps,
                        lhsT=anorm[:, tok0 : tok0 + P],
                        rhs=wout_sb[:, e2 * 512 : (e2 + 1) * 512],
                        start=True,
                        stop=True,
                    )
                    ob = outst.tile([P, 512], f32, tag="outst")
                    nc.vector.tensor_copy(out=ob, in_=ps)
                    nc.sync.dma_start(
                        out=out[tok0 : tok0 + P, e2 * 512 : (e2 + 1) * 512], in_=ob
                    )

        # ---- program order: overlap batch-1 QKV with batch-0 attention ----
        for t in range(4):
            qkv_chunk(t)
        vtrans(0)
        extra = {(0, 0): [4], (0, 1): [5], (0, 2): [6], (0, 3): [7]}
        for b in range(B):
            for qq in range(QQ):
                attention_unit(b, qq)
                for t in extra.get((b, qq), []):
                    qkv_chunk(t)
                if (b, qq) == (0, QQ - 1):
                    vtrans(1)
                outproj_unit(b, qq)

        if dbg:
            nc.sync.dma_start(out=dbg["kq"][0], in_=k_sb[:, :])
            nc.sync.dma_start(out=dbg["kq"][1], in_=q_sb[:, :])
            nc.sync.dma_start(out=dbg["vt"][:, :], in_=vt_sb[:, :])
            nc.sync.dma_start(
                out=dbg["vaug"][:, :],
                in_=vaug.rearrange("p b h c d -> p (b h c d)"),
            )
            nc.sync.dma_start(out=dbg["anorm"][:, :], in_=anorm[:, :])


def _get_graph():
    if "nc" not in _CACHE:
        _CACHE["nc"] = _build_graph()
    return _CACHE["nc"]


def kernel(**inputs):
    x = np.asarray(inputs["x"], dtype=np.float32)
    W_qkv = np.asarray(inputs["W_qkv"], dtype=np.float32)
    b_qkv = np.asarray(inputs["b_qkv"], dtype=np.float32)
    W_out = np.asarray(inputs["W_out"], dtype=np.float32)
    b_out = np.asarray(inputs["b_out"], dtype=np.float32)

    nc = _get_graph()

    bf16 = ml_dtypes.bfloat16
    xT = np.ascontiguousarray(x.reshape(TOK, EMB).T).astype(bf16)
    in_maps = []
    for c in range(8):
        cols = np.concatenate(
            [
                np.arange(c * 128, (c + 1) * 128),
                np.arange(1024 + c * 128, 1024 + (c + 1) * 128),
                np.arange(2048 + c * 128, 2048 + (c + 1) * 128),
            ]
        )
        in_maps.append(
            {
                "xT": xT,
                "wqkv": np.ascontiguousarray(W_qkv[:, cols]).astype(bf16),
                "bqkv": np.ascontiguousarray(b_qkv[cols]).astype(np.float32),
                "wout": np.ascontiguousarray(
                    W_out[c * 128 : (c + 1) * 128, :]
                ).astype(bf16),
            }
        )

    from concourse.bass_utils import run_bass_kernel_spmd

    res = run_bass_kernel_spmd(nc, in_maps, core_ids=list(range(8)))
    LAST["results"] = res

    acc = np.zeros((TOK, EMB), np.float32)
    for r in res.results:
        acc += r["out"]
    acc += b_out[None, :]
    return acc.reshape(B, N, EMB).astype(np.float32)


if __name__ == "__main__":
    # smoke test with random inputs
    rng = np.random.default_rng(0)
    inputs = {
        "x": rng.standard_normal((B, N, EMB), dtype=np.float32),
        "W_qkv": rng.standard_normal((EMB, 3072), dtype=np.float32) * EMB**-0.5,
        "b_qkv": np.zeros((3072,), np.float32),
        "W_out": rng.standard_normal((1024, EMB), dtype=np.float32) * 1024**-0.5,
        "b_out": np.zeros((EMB,), np.float32),
    }
    y = kernel(**inputs)
    print("out", y.shape, y.dtype, float(np.abs(y).mean()))


# revision 17
# speedup vs baseline: 1.0298x; 1.0298x over previous
"""Multi-head attention (B=2, N=2048, EMB=1024, H=16, hd=64) on 8 TRN2 NeuronCores.

Sharding: tensor-parallel over heads. Each core owns 2 heads: it gets the
W_qkv columns (k|q|v sections) and W_out rows for those heads, computes
QKV projection + attention + its partial output projection, and the host
sums the 8 partials (the "all-reduce") and adds b_out.

Device kernel layout (per core), all matmuls bf16 with fp32 PSUM accumulation:
  - x is pre-transposed on host to xT [EMB, TOK] so the embedding dim lands on
    SBUF partitions (matmul contraction dim).
  - QKV^T is produced in [dims, tokens] layout: lhsT = W chunk, rhs = xT chunk.
    K/Q sections stay transposed ([hd, tok]) for the scores matmul; the V
    section is staged transposed then PE-transposed into [tok, hd(+ones)]
    tiles (V_aug) for the attn@V matmul.
  - scores^T chunks [k_tok 128, q 512] per head via row-packed (K=64) matmuls,
    exp on ScalarE straight out of PSUM (scale=1/8 folded in, no max
    subtraction needed: scores ~ N(0,1)), bf16 expT.
  - attn@V: lhsT = V_aug [k_tok 128, 65] (col 64 = ones -> row 64 of the
    output accumulates the softmax denominator), accumulated over 16 k chunks.
  - normalize: reciprocal of the denominator row, partition-broadcast DMA,
    multiply on VectorE into A_norm [att 128, tok] bf16.
  - out projection: lhsT = A_norm chunk [128, 128], rhs = W_out shard
    [128, 512], PSUM -> SBUF -> DRAM partial [TOK, EMB] fp32.
"""

import os

import numpy as np
import ml_dtypes

B = 2
N = 2048
EMB = 1024
TOK = B * N  # 4096
HD = 64
H_PER_CORE = 2
DIMS = 3 * H_PER_CORE * HD  # 384 qkv cols per core
ATT_LOCAL = H_PER_CORE * HD  # 128
P = 128
EC = EMB // P  # 8 embedding chunks
TCQ = TOK // 512  # 8 token chunks for the qkv projection
KCH = N // P  # 16 key chunks per batch
QQ = N // 512  # 4 query quarters per batch
SCALE = HD ** -0.5

_CACHE = {}
LAST = {}


def _build_graph():
    from concourse import bacc, mybir
    import concourse.tile as tile

    nc = bacc.Bacc(
        "TRN2", target_bir_lowering=False, debug=False, num_devices=8
    )
    dt = mybir.dt
    xT = nc.dram_tensor("xT", [EMB, TOK], dt.bfloat16, kind="ExternalInput")
    wqkv = nc.dram_tensor("wqkv", [EMB, DIMS], dt.bfloat16, kind="ExternalInput")
    bqkv = nc.dram_tensor("bqkv", [DIMS], dt.float32, kind="ExternalInput")
    wout = nc.dram_tensor("wout", [ATT_LOCAL, EMB], dt.bfloat16, kind="ExternalInput")
    out = nc.dram_tensor("out", [TOK, EMB], dt.float32, kind="ExternalOutput")

    dbg = {}
    if os.environ.get("KERNEL_DEBUG") == "1":
        dbg["kq"] = nc.dram_tensor(
            "dbg_kq", [2, P, TOK], dt.bfloat16, kind="ExternalOutput"
        )
        dbg["vt"] = nc.dram_tensor(
            "dbg_vt", [P, TOK], dt.float32, kind="ExternalOutput"
        )
        dbg["vaug"] = nc.dram_tensor(
            "dbg_vaug", [P, B * H_PER_CORE * KCH * P], dt.bfloat16,
            kind="ExternalOutput",
        )
        dbg["exp"] = nc.dram_tensor(
            "dbg_exp", [P, 1024], dt.bfloat16, kind="ExternalOutput"
        )
        dbg["psa"] = nc.dram_tensor(
            "dbg_psa", [2, HD + 1, 512], dt.float32, kind="ExternalOutput"
        )
        dbg["rrep"] = nc.dram_tensor(
            "dbg_rrep", [2, HD, 512], dt.float32, kind="ExternalOutput"
        )
        dbg["anorm"] = nc.dram_tensor(
            "dbg_anorm", [P, TOK], dt.bfloat16, kind="ExternalOutput"
        )

    with tile.TileContext(nc) as tc:
        _emit(tc, nc, xT, wqkv, bqkv, wout, out, dbg)
    nc.compile()
    return nc


def _emit(tc, nc, xT, wqkv, bqkv, wout, out, dbg=None):
    dbg = dbg or {}
    from contextlib import ExitStack
    import concourse.bass as bass
    from concourse import mybir
    from concourse.masks import make_identity

    dt = mybir.dt
    f32, bf16 = dt.float32, dt.bfloat16
    Exp = mybir.ActivationFunctionType.Exp

    with ExitStack() as ctx:
        consts = ctx.enter_context(tc.tile_pool(name="consts", bufs=1))
        xt_pool = ctx.enter_context(tc.tile_pool(name="xt", bufs=2))
        persist = ctx.enter_context(tc.tile_pool(name="persist", bufs=1))
        expp = ctx.enter_context(tc.tile_pool(name="expp", bufs=6))
        small = ctx.enter_context(tc.tile_pool(name="small", bufs=2))
        dram_p = ctx.enter_context(tc.tile_pool(name="dram_p", bufs=2, space="DRAM"))
        outst = ctx.enter_context(tc.tile_pool(name="outst", bufs=3))
        ps_scores = ctx.enter_context(
            tc.tile_pool(name="ps_scores", bufs=2, space="PSUM")
        )
        ps_att = ctx.enter_context(tc.tile_pool(name="ps_att", bufs=2, space="PSUM"))
        ps_small = ctx.enter_context(
            tc.tile_pool(name="ps_small", bufs=2, space="PSUM")
        )

        # ---- constants / persistent tiles ----
        # warm up the exp table set as early as possible (one-time ~2.7us)
        warm = consts.tile([1, 8], f32, tag="warm")
        nc.vector.memset(warm, 0.0)
        nc.scalar.activation(out=warm, in_=warm, func=Exp, scale=1.0)

        w_sb = consts.tile([P, EC, DIMS], bf16, tag="w_sb")
        for e in range(EC):
            nc.sync.dma_start(out=w_sb[:, e, :], in_=wqkv[e * P : (e + 1) * P, :])
        bias_sb = consts.tile([P, 3], f32, tag="bias_sb")
        nc.sync.dma_start(out=bias_sb, in_=bqkv[:].rearrange("(c p) -> p c", p=P))
        wout_sb = consts.tile([P, EMB], bf16, tag="wout_sb")
        nc.sync.dma_start(out=wout_sb, in_=wout[:, :])
        ident = consts.tile([P, P], f32, tag="ident")
        make_identity(nc, ident)

        k_sb = persist.tile([P, TOK], bf16, tag="k_sb")
        q_sb = persist.tile([P, TOK], bf16, tag="q_sb")
        vt_sb = persist.tile([P, TOK], f32, tag="vt_sb")
        # padded to 128 columns so the attn@V weight load gets FWL
        vaug = persist.tile([P, B, H_PER_CORE, KCH, P], bf16, tag="vaug")
        anorm = persist.tile([P, TOK], bf16, tag="anorm")
        nc.vector.memset(vaug[:, :, :, :, :], 0.0)
        # ones column of V_aug (the softmax denominator accumulator row)
        nc.vector.memset(vaug[:, :, :, :, HD : HD + 1], 1.0)

        qkv_dst = (k_sb, q_sb, vt_sb)

        def qkv_chunk(t):
            # tokens t*512 .. (t+1)*512
            xt = xt_pool.tile([P, EC, 512], bf16, tag="xt")
            for e in range(EC):
                nc.sync.dma_start(
                    out=xt[:, e, :], in_=xT[e * P : (e + 1) * P, bass.ts(t, 512)]
                )
            for d in range(3):
                ps = ps_small.tile([P, 512], f32, tag="ps_small")
                for e in range(EC):
                    nc.tensor.matmul(
                        ps,
                        lhsT=w_sb[:, e, d * P : (d + 1) * P],
                        rhs=xt[:, e, :],
                        start=(e == 0),
                        stop=(e == EC - 1),
                    )
                nc.vector.tensor_scalar_add(
                    out=qkv_dst[d][:, bass.ts(t, 512)],
                    in0=ps,
                    scalar1=bias_sb[:, d : d + 1],
                )

        def vtrans(b):
            # fill vaug[:, b, h, i, 0:64] = V[tok chunk i, head h] for batch b
            for i in range(KCH):
                base = b * N + i * P
                for h in range(H_PER_CORE):
                    ps = ps_small.tile([P, 512], f32, tag="ps_small")
                    nc.tensor.transpose(
                        ps[:, 0:HD],
                        in_=vt_sb[h * HD : (h + 1) * HD, base : base + P],
                        identity=ident[h * HD : (h + 1) * HD, h * HD : (h + 1) * HD],
                    )
                    nc.vector.tensor_copy(
                        out=vaug[:, b, h, i, 0:HD], in_=ps[:, 0:HD]
                    )

        def attention_unit(b, qq):
            # query tokens qbase .. qbase+512 of batch b, both heads
            qbase = b * N + qq * 512
            ps_a = [
                ps_att.tile([P, 512], f32, tag="ps_att", name=f"ps_a{b}_{qq}_{h}")
                for h in range(H_PER_CORE)
            ]
            for i in range(KCH):
                kbase = b * N + i * P
                ps_s = ps_scores.tile([P, 1024], f32, tag="ps_s")
                for h in range(H_PER_CORE):
                    nc.tensor.matmul(
                        ps_s[:, h * 512 : (h + 1) * 512],
                        lhsT=k_sb[h * HD : (h + 1) * HD, kbase : kbase + P],
                        rhs=q_sb[h * HD : (h + 1) * HD, qbase : qbase + 512],
                        start=True,
                        stop=True,
                    )
                ex = expp.tile([P, 1024], bf16, tag="expT")
                nc.scalar.activation(out=ex, in_=ps_s, func=Exp, scale=SCALE)
                if "exp" in dbg and (b, qq, i) == (0, 0, 0):
                    nc.sync.dma_start(out=dbg["exp"][:, :], in_=ex)
                for h in range(H_PER_CORE):
                    nc.tensor.matmul(
                        ps_a[h],
                        lhsT=vaug[:, b, h, i, :],
                        rhs=ex[:, h * 512 : (h + 1) * 512],
                        start=(i == 0),
                        stop=(i == KCH - 1),
                    )
            for h in range(H_PER_CORE):
                # evict PSUM -> SBUF immediately so the accumulator slot frees
                # fast; all normalization then runs off the critical path.
                psa_sb = small.tile([HD + 1, 512], f32, tag="psa_sb")
                nc.vector.tensor_copy(out=psa_sb, in_=ps_a[h][0 : HD + 1, :])
                if "psa" in dbg and (b, qq) == (0, 0):
                    nc.sync.dma_start(out=dbg["psa"][h], in_=psa_sb)
                rc = small.tile([HD + 1, 512], f32, tag="recip")
                nc.vector.reciprocal(
                    out=rc[HD : HD + 1, :], in_=psa_sb[HD : HD + 1, :]
                )
                # partition-broadcast via a DRAM bounce: SBUF-source DMAs
                # reject partition-step-0 APs, DRAM-source ones allow it.
                rdram = dram_p.tile([1, 512], f32, tag="rdram")
                nc.sync.dma_start(out=rdram[:, :], in_=rc[HD : HD + 1, :])
                rrep = small.tile([HD, 512], f32, tag="rrep")
                nc.sync.dma_start(
                    out=rrep, in_=rdram[0:1, :].to_broadcast((HD, 512))
                )
                if "rrep" in dbg and (b, qq) == (0, 0):
                    nc.sync.dma_start(out=dbg["rrep"][h], in_=rrep)
                if h == 0:
                    nc.vector.tensor_mul(
                        out=anorm[0:HD, qbase : qbase + 512],
                        in0=psa_sb[0:HD, :],
                        in1=rrep,
                    )
                else:
                    # engine lanes cannot shift partitions; go through a
                    # partition-0 temp and DMA into partitions 64..127.
                    tmp = small.tile([HD, 512], bf16, tag="anorm_tmp")
                    nc.vector.tensor_mul(out=tmp, in0=psa_sb[0:HD, :], in1=rrep)
                    nc.sync.dma_start(
                        out=anorm[HD : 2 * HD, qbase : qbase + 512], in_=tmp
                    )

        def outproj_unit(b, qq):
            qbase = b * N + qq * 512
            for tci in range(4):
                tok0 = qbase + tci * P
                for e2 in range(2):
                    ps = ps_small.tile([P, 512], f32, tag="ps_small")
                    nc.tensor.matmul(
                        ps,
                        lhsT=anorm[:, tok0 : tok0 + P],
                        rhs=wout_sb[:, e2 * 512 : (e2 + 1) * 512],
                        start=True,
                        stop=True,
                    )
                    ob = outst.tile([P, 512], f32, tag="outst")
                    nc.vector.tensor_copy(out=ob, in_=ps)
                    nc.sync.dma_start(
                        out=out[tok0 : tok0 + P, e2 * 512 : (e2 + 1) * 512], in_=ob
                    )

        # ---- program order: overlap batch-1 QKV with batch-0 attention ----
        for t in range(4):
            qkv_chunk(t)
        vtrans(0)
        extra = {(0, 0): [4], (0, 1): [5], (0, 2): [6], (0, 3): [7]}
        for b in range(B):
            for qq in range(QQ):
                attention_unit(b, qq)
                for t in extra.get((b, qq), []):
                    qkv_chunk(t)
                if (b, qq) == (0, QQ - 1):
                    vtrans(1)
                outproj_unit(b, qq)

        if dbg:
            nc.sync.dma_start(out=dbg["kq"][0], in_=k_sb[:, :])
            nc.sync.dma_start(out=dbg["kq"][1], in_=q_sb[:, :])
            nc.sync.dma_start(out=dbg["vt"][:, :], in_=vt_sb[:, :])
            nc.sync.dma_start(
                out=dbg["vaug"][:, :],
                in_=vaug.rearrange("p b h c d -> p (b h c d)"),
            )
            nc.sync.dma_start(out=dbg["anorm"][:, :], in_=anorm[:, :])


def _get_graph():
    if "nc" not in _CACHE:
        _CACHE["nc"] = _build_graph()
    return _CACHE["nc"]


def kernel(**inputs):
    x = np.asarray(inputs["x"], dtype=np.float32)
    W_qkv = np.asarray(inputs["W_qkv"], dtype=np.float32)
    b_qkv = np.asarray(inputs["b_qkv"], dtype=np.float32)
    W_out = np.asarray(inputs["W_out"], dtype=np.float32)
    b_out = np.asarray(inputs["b_out"], dtype=np.float32)

    nc = _get_graph()

    bf16 = ml_dtypes.bfloat16
    xT = np.ascontiguousarray(x.reshape(TOK, EMB).T).astype(bf16)
    in_maps = []
    for c in range(8):
        cols = np.concatenate(
            [
                np.arange(c * 128, (c + 1) * 128),
                np.arange(1024 + c * 128, 1024 + (c + 1) * 128),
                np.arange(2048 + c * 128, 2048 + (c + 1) * 128),
            ]
        )
        in_maps.append(
            {
                "xT": xT,
                "wqkv": np.ascontiguousarray(W_qkv[:, cols]).astype(bf16),
                "bqkv": np.ascontiguousarray(b_qkv[cols]).astype(np.float32),
                "wout": np.ascontiguousarray(
                    W_out[c * 128 : (c + 1) * 128, :]
                ).astype(bf16),
            }
        )

    from concourse.bass_utils import run_bass_kernel_spmd

    res = run_bass_kernel_spmd(nc, in_maps, core_ids=list(range(8)))
    LAST["results"] = res

    acc = np.zeros((TOK, EMB), np.float32)
    for r in res.results:
        acc += r["out"]
    acc += b_out[None, :]
    return acc.reshape(B, N, EMB).astype(np.float32)


if __name__ == "__main__":
    # smoke test with random inputs
    rng = np.random.default_rng(0)
    inputs = {
        "x": rng.standard_normal((B, N, EMB), dtype=np.float32),
        "W_qkv": rng.standard_normal((EMB, 3072), dtype=np.float32) * EMB**-0.5,
        "b_qkv": np.zeros((3072,), np.float32),
        "W_out": rng.standard_normal((1024, EMB), dtype=np.float32) * 1024**-0.5,
        "b_out": np.zeros((EMB,), np.float32),
    }
    y = kernel(**inputs)
    print("out", y.shape, y.dtype, float(np.abs(y).mean()))
